# revision 1
# baseline (speedup 1.0000x reference)
"""GQA attention (dense_transformer) TRN2 Bass kernel — 8 NeuronCores.

Problem: b=2, s=2048, d=2048, nh=16, nkv=4, hd=128, causal GQA attention
block with RMS-normed+RoPE'd q/k and per-head q gains.

Sharding: batch DP=2 x head TP=4  ->  8 cores. Each core handles one batch
element, 4 q heads, 1 kv head. Wq/Wk/Wv column-sharded, Wo row-sharded;
partial outputs summed on host.

Per-core dataflow (matmuls bf16 in / fp32 PSUM), three pipelined phases:
  1. Projections: per 128-row s-tile, Q and fused-KV matmuls accumulate
     over d; PSUM copied straight to SBUF (fast PSUM release); RMS-norm
     scale computed on DVE only (bit-trick rsqrt + 2 Newton steps, no ACT
     table switches); norm+qg folded into RoPE via scalar_tensor_tensor;
     PE-transpose Q,K into [hd, s] layout.
  2. Attention per (head, 512-wide q-chunk, 128-wide k-tile): S^T =
     K-tile @ Q-chunk on PE; exp on ScalarE with 1/sqrt(hd) folded in;
     causal zeroing of diagonal tiles on gpsimd (affine_select); A@V and
     replicated row-sum as accumulating PE matmuls (k-loop software-
     pipelined so PE never waits on exp); fast-reciprocal + multiply
     normalize.
  3. Output projection -> outT partial [d, s] fp32; host transposes and
     sums the TP partials.
"""

import math
import sys

if "/opt/trn_rl_repo" not in sys.path:
    sys.path.insert(0, "/opt/trn_rl_repo")

import numpy as np
import ml_dtypes

import concourse.mybir as mybir
import concourse.tile as tile
from concourse.bass_types import AP
from concourse import bacc
from concourse.bass import _add_dep_helper
from concourse.bass_utils import run_bass_kernel_spmd

F32 = mybir.dt.float32
I32 = mybir.dt.int32
BF16 = mybir.dt.bfloat16
AF = mybir.ActivationFunctionType
ALU = mybir.AluOpType

BF16NP = ml_dtypes.bfloat16
RMS_EPS = float(np.finfo(np.float32).eps)

S, D, NQ, HD = 2048, 2048, 4, 128
DQ = NQ * HD            # 512: per-core q width
NTP = 4                 # tensor-parallel ways (heads)
NB = 2                  # batch (data-parallel ways)
NCORES = 8

_NC_CACHE = {}


def build_kernel(S=S, D=D, NQ=NQ, HD=HD, num_devices=NCORES):
    DQ = NQ * HD
    NST = S // 128          # s-tiles
    NDC = D // 128          # d-chunks (projection contraction)
    NQC = S // 512          # q-chunks for attention
    scale = 1.0 / math.sqrt(HD)
    H = HD // 2

    nc = bacc.Bacc("TRN2", target_bir_lowering=False, debug=False,
                   num_devices=num_devices)

    xT = nc.dram_tensor("xT", [D, S], BF16, kind="ExternalInput").ap()
    wq = nc.dram_tensor("wq", [D, DQ], BF16, kind="ExternalInput").ap()
    wkv = nc.dram_tensor("wkv", [D, 2 * HD], BF16, kind="ExternalInput").ap()
    wo = nc.dram_tensor("wo", [DQ, D], BF16, kind="ExternalInput").ap()
    cst = nc.dram_tensor("cst", [S, 2 * HD], F32, kind="ExternalInput").ap()
    qgb = nc.dram_tensor("qgb", [128, 4 * (NQ + 1)], F32,
                         kind="ExternalInput").ap()
    ident = nc.dram_tensor("ident", [128, 128], BF16, kind="ExternalInput").ap()
    msk = nc.dram_tensor("msk", [128, 4 * 512], BF16, kind="ExternalInput").ap()
    ones = nc.dram_tensor("ones", [128, 128], BF16, kind="ExternalInput").ap()
    outT = nc.dram_tensor("outT", [D, S], F32, kind="ExternalOutput").ap()

    with tile.TileContext(nc) as tc:
        from contextlib import ExitStack
        with ExitStack() as ctx:
            consts = ctx.enter_context(tc.tile_pool(name="consts", bufs=1))
            xpool = ctx.enter_context(tc.tile_pool(name="xT", bufs=1))
            wpool = ctx.enter_context(tc.tile_pool(name="w", bufs=1))
            qt_pool = ctx.enter_context(tc.tile_pool(name="qt", bufs=1))
            yt_pool = ctx.enter_context(tc.tile_pool(name="yt", bufs=1))
            v_pool = ctx.enter_context(tc.tile_pool(name="vrow", bufs=1))

            # ---- input DMAs: weights first, then xT in dependency-staggered
            # waves so the first chunks get full HBM bandwidth (the first
            # matmuls start ~25us earlier than with all DMAs in flight at
            # once), then rope tables, wo (needed last) at the end.
            wq_sb = wpool.tile([128, NDC, DQ], BF16, tag="wq")
            nc.sync.dma_start(wq_sb[:], wq.rearrange("(n p) m -> p n m", p=128))
            wkv_sb = wpool.tile([128, NDC, 2 * HD], BF16, tag="wkv")
            nc.sync.dma_start(wkv_sb[:], wkv.rearrange("(n p) m -> p n m", p=128))

            xT_sb = xpool.tile([128, NDC, S], BF16, tag="xT")
            xTr = xT.rearrange("(n p) m -> p n m", p=128)
            x_dmas = []
            for dc in range(NDC):
                dma = nc.sync.dma_start(xT_sb[:, dc, :], xTr[:, dc, :])
                if dc >= 4:
                    _add_dep_helper(dma.ins, x_dmas[dc - 4].ins, sync=True,
                                    reason="stagger xT input waves")
                x_dmas.append(dma)

            cst_sb = consts.tile([128, NST, 2 * HD], F32, tag="cst")
            d_cc = nc.sync.dma_start(cst_sb[:],
                                     cst.rearrange("(n p) m -> p n m", p=128))
            _add_dep_helper(d_cc.ins, x_dmas[min(3, NDC - 1)].ins, sync=True,
                            reason="after wave0")
            qgb_sb = consts.tile([128, 4 * (NQ + 1)], F32, tag="qgb")
            nc.sync.dma_start(qgb_sb[:], qgb)
            ident_sb = consts.tile([128, 128], BF16, tag="ident")
            nc.sync.dma_start(ident_sb[:], ident)
            ones_sb = consts.tile([128, 128], BF16, tag="ones")
            nc.sync.dma_start(ones_sb[:], ones)
            msk_sb = consts.tile([128, 4, 512], BF16, tag="msk")
            nc.sync.dma_start(msk_sb[:], msk.rearrange("p (m c) -> p m c", c=512))

            wo_sb = wpool.tile([128, NQ, D], BF16, tag="wo")
            d_wo = nc.sync.dma_start(wo_sb[:], wo.rearrange("(n p) m -> p n m", p=128))
            _add_dep_helper(d_wo.ins, x_dmas[min(11, NDC - 1)].ins, sync=True,
                            reason="wo last")

            qt_all = qt_pool.tile([128, NQ + 1, S], BF16, name="qt_all",
                                  tag="qt_all")
            yt_tiles = [yt_pool.tile([128, S], BF16, name=f"yt{h}", tag=f"yt{h}")
                        for h in range(NQ)]
            v_tiles = [v_pool.tile([128, HD], BF16, name=f"v{st}", tag=f"v{st}")
                       for st in range(NST)]

            # ---- Phase 1: projections + rms-norm + rope + transpose ----
            with (
                tc.tile_pool(name="pqkv", bufs=2, space="PSUM") as pqkv,
                tc.tile_pool(name="ptr", bufs=2, space="PSUM") as ptr,
                tc.tile_pool(name="p1qkv", bufs=2) as p1qkv,
                tc.tile_pool(name="p1tmp", bufs=3) as p1tmp,
                tc.tile_pool(name="p1stat", bufs=3) as p1stat,
            ):
                NH1 = NQ + 1
                for st in range(NST):
                    pq = pqkv.tile([128, DQ], F32, tag="pq")
                    pkv = pqkv.tile([128, 2 * HD], F32, tag="pkv")
                    for dc in range(NDC):
                        lhsT = xT_sb[:, dc, st * 128:(st + 1) * 128]
                        first, last = dc == 0, dc == NDC - 1
                        nc.tensor.matmul(pq[:], lhsT, wq_sb[:, dc, :],
                                         start=first, stop=last)
                        nc.tensor.matmul(pkv[:], lhsT, wkv_sb[:, dc, :],
                                         start=first, stop=last)

                    # copy PSUM->SBUF immediately: releases the accumulators
                    # for the next s-tile and lets DVE run in 2x fp32 mode.
                    qsb = p1qkv.tile([128, DQ], F32, tag="qsb")
                    nc.scalar.copy(qsb[:], pq[:])
                    kvsb = p1qkv.tile([128, 2 * HD], F32, tag="kvsb")
                    nc.scalar.copy(kvsb[:], pkv[:])
                    nc.vector.tensor_copy(v_tiles[st][:], kvsb[:, HD:2 * HD])

                    sq_scratch = p1tmp.tile([128, HD], F32, tag="sqs")
                    ssq = p1stat.tile([128, NH1], F32, tag="ssq")
                    for i in range(NH1):
                        src = (qsb[:, i * HD:(i + 1) * HD] if i < NQ
                               else kvsb[:, 0:HD])
                        nc.scalar.activation(sq_scratch[:], src, AF.Square,
                                             accum_out=ssq[:, i:i + 1])
                    # rinv = (mean(q^2)+eps)**-0.5 on DVE only (bit-trick
                    # seed + 2 Newton steps) — keeps ScalarE on a single
                    # table set for the whole kernel; q lanes of qgb carry
                    # qg, the k lane carries 1.0.
                    m = p1stat.tile([128, NH1], F32, tag="m")
                    nc.vector.tensor_scalar(m[:], ssq[:], 1.0 / HD, RMS_EPS,
                                            op0=ALU.mult, op1=ALU.add)
                    y0 = p1stat.tile([128, NH1], F32, tag="y0")
                    nc.vector.tensor_scalar(y0[:].bitcast(I32),
                                            m[:].bitcast(I32), 1, None,
                                            op0=ALU.arith_shift_right)
                    nc.vector.tensor_scalar(y0[:].bitcast(I32),
                                            y0[:].bitcast(I32),
                                            -1, 0x5F3759DF,
                                            op0=ALU.mult, op1=ALU.add)
                    rinv = y0
                    for _ in range(2):
                        aa = p1stat.tile([128, NH1], F32, tag="nr_a")
                        nc.vector.tensor_mul(aa[:], rinv[:], rinv[:])
                        nc.vector.tensor_mul(aa[:], aa[:], m[:])
                        nc.vector.tensor_scalar(aa[:], aa[:], -0.5, 1.5,
                                                op0=ALU.mult, op1=ALU.add)
                        nxt = p1stat.tile([128, NH1], F32, tag="nr_y")
                        nc.vector.tensor_mul(nxt[:], rinv[:], aa[:])
                        rinv = nxt
                    nc.vector.tensor_mul(rinv[:], rinv[:], qgb_sb[:, 0:NH1])

                    cst_t = cst_sb[:, st, :]
                    pt5 = ptr.tile([128, NH1 * 128], BF16, tag="ptr")
                    # per head: one pass computes [t | v] = (q*r) * [c|c|-s|s]
                    # (q replicated via a stride-0 AP) into a slice of tv5;
                    # then ONE batched add over all heads applies the rope
                    # half-swap via the second operand's AP.
                    tv5 = p1tmp.tile([128, NH1 * 2 * HD], F32, tag="ropetv5")
                    for i in range(NH1):
                        q_ap = (qsb[:, i * HD:(i + 1) * HD] if i < NQ
                                else kvsb[:, 0:HD])
                        q_rep = AP(q_ap.tensor, q_ap.offset,
                                   [q_ap.ap[0], [0, 2], [1, HD]])
                        nc.vector.scalar_tensor_tensor(
                            tv5[:, i * 2 * HD:(i + 1) * 2 * HD],
                            q_rep, rinv[:, i:i + 1], cst_t,
                            op0=ALU.mult, op1=ALU.mult)
                    ro5 = p1tmp.tile([128, NH1 * HD], BF16, tag="ro5")
                    b5 = tv5[:]
                    t_view = AP(b5.tensor, b5.offset,
                                [b5.ap[0], [2 * HD, NH1], [H, 2], [1, H]])
                    v_view = AP(b5.tensor, b5.offset + HD + H,
                                [b5.ap[0], [2 * HD, NH1], [-H, 2], [1, H]])
                    r5 = ro5[:]
                    o_view = AP(r5.tensor, r5.offset,
                                [r5.ap[0], [HD, NH1], [H, 2], [1, H]])
                    nc.vector.tensor_add(o_view, t_view, v_view)
                    for i in range(NH1):
                        nc.tensor.transpose(pt5[:, i * 128:(i + 1) * 128],
                                            ro5[:, i * HD:(i + 1) * HD],
                                            ident_sb[:])
                    nc.scalar.copy(
                        qt_all[:, :, st * 128:(st + 1) * 128],
                        pt5[:].rearrange("p (h c) -> p h c", c=128))

            # ---- Phase 2: attention (k-loop software-pipelined 2 deep) ----
            with (
                tc.tile_pool(name="ps", bufs=4, space="PSUM") as ps_pool,
                tc.tile_pool(name="py", bufs=2, space="PSUM") as py_pool,
                tc.tile_pool(name="pr", bufs=2, space="PSUM") as pr_pool,
                tc.tile_pool(name="ptile", bufs=5) as pt_pool,
                tc.tile_pool(name="rcp", bufs=2) as rcp_pool,
            ):
                kt_row = qt_all[:, NQ, :]
                for qc in range(NQC):
                    n_kt = 4 * qc + 4
                    for h in range(NQ):
                        qs = qt_all[:, h, qc * 512:(qc + 1) * 512]
                        py = py_pool.tile([128, 512], F32, tag="py")
                        pr = pr_pool.tile([128, 512], F32, tag="pr")

                        def tile_off(kt):
                            # diagonal tile m=kt-4qc only has surviving
                            # (q >= k) elements at q-columns >= 128m: narrow
                            # every op on it to [128m, 512).
                            return max(0, kt - 4 * qc) * 128 if kt >= 4 * qc else 0

                        def emit_scores(kt):
                            off = tile_off(kt)
                            pss = ps_pool.tile([128, 512], F32, name="ps",
                                               tag="ps")
                            nc.tensor.matmul(
                                pss[:, off:512],
                                kt_row[:, kt * 128:(kt + 1) * 128],
                                qs[:, off:512], start=True, stop=True)
                            pt = pt_pool.tile([128, 512], BF16, name="pt",
                                              tag="pt")
                            nc.scalar.activation(pt[:, off:512],
                                                 pss[:, off:512], AF.Exp,
                                                 scale=scale)
                            if kt >= 4 * qc:  # diagonal: zero where q < k
                                m = kt - 4 * qc
                                nc.vector.tensor_mul(
                                    pt[:, off:512], pt[:, off:512],
                                    msk_sb[:, m, off:512])
                            return pt

                        def emit_av(kt, pt, first, last):
                            off = tile_off(kt)
                            nc.tensor.matmul(py[:, off:512], v_tiles[kt][:],
                                             pt[:, off:512],
                                             start=first, stop=last)
                            nc.tensor.matmul(pr[:, off:512], ones_sb[:],
                                             pt[:, off:512],
                                             start=first, stop=last)

                        pts = [emit_scores(0), emit_scores(1)]
                        for kt in range(2, n_kt):
                            pts.append(emit_scores(kt))
                            emit_av(kt - 2, pts[kt - 2], kt == 2, False)
                        emit_av(n_kt - 2, pts[n_kt - 2], False, False)
                        emit_av(n_kt - 1, pts[n_kt - 1], False, True)

                        rcp = rcp_pool.tile([128, 512], F32, tag="rcp")
                        nc.vector.reciprocal_approx_fast(out=rcp[:], in_=pr[:])
                        nc.vector.tensor_mul(
                            yt_tiles[h][:, qc * 512:(qc + 1) * 512],
                            py[:], rcp[:])

            # ---- Phase 3: output projection ----
            with (
                tc.tile_pool(name="po", bufs=4, space="PSUM") as po_pool,
                tc.tile_pool(name="ob", bufs=4) as ob_pool,
            ):
                for dt in range(NDC):
                    for qc in range(NQC):
                        po = po_pool.tile([128, 512], F32, tag="po")
                        for dqc in range(NQ):
                            nc.tensor.matmul(
                                po[:], wo_sb[:, dqc, dt * 128:(dt + 1) * 128],
                                yt_tiles[dqc][:, qc * 512:(qc + 1) * 512],
                                start=(dqc == 0), stop=(dqc == NQ - 1))
                        ob = ob_pool.tile([128, 512], F32, tag="ob")
                        if (dt + qc) % 2 == 0:
                            nc.scalar.copy(ob[:], po[:])
                        else:
                            nc.vector.tensor_copy(ob[:], po[:])
                        nc.sync.dma_start(
                            outT[dt * 128:(dt + 1) * 128,
                                 qc * 512:(qc + 1) * 512], ob[:])

    nc.compile()
    return nc


def get_nc():
    if "nc" not in _NC_CACHE:
        _NC_CACHE["nc"] = build_kernel()
    return _NC_CACHE["nc"]


def rope_tables(S=S, HD=HD):
    """Packed rope table [S, 2*HD]: [c | c | -s | s]."""
    f = 1.0 / (10000.0 ** (np.arange(0, HD, 2, dtype=np.float32) / HD))
    fr = np.outer(np.arange(S, dtype=np.float32), f)
    c = np.cos(fr).astype(np.float32)
    s = np.sin(fr).astype(np.float32)
    return np.concatenate([c, c, -s, s], axis=1)


def make_in_maps(x, Wq, Wk, Wv, Wo, qg):
    x = np.asarray(x, np.float32)
    Wq = np.asarray(Wq, np.float32)
    Wk = np.asarray(Wk, np.float32)
    Wv = np.asarray(Wv, np.float32)
    Wo = np.asarray(Wo, np.float32)
    qg = np.asarray(qg, np.float32)
    cst = rope_tables()
    ident = np.eye(128, dtype=BF16NP)
    ones = np.ones((128, 128), dtype=BF16NP)
    pp, ff = np.arange(128)[:, None], np.arange(512)[None, :]
    msk = np.concatenate(
        [(ff >= pp + 128 * mm).astype(BF16NP) for mm in range(4)], axis=1)
    xT = [np.ascontiguousarray(x[b].T).astype(BF16NP) for b in range(NB)]
    in_maps = []
    for b in range(NB):
        for tp in range(NTP):
            qgb_row = np.broadcast_to(
                np.concatenate([qg[tp * NQ:(tp + 1) * NQ],
                                [np.float32(1.0)]] * 4)[None, :].astype(np.float32),
                (128, 4 * (NQ + 1))).copy()
            wkv = np.concatenate([
                Wk[tp * HD:(tp + 1) * HD, :].T,
                Wv[tp * HD:(tp + 1) * HD, :].T], axis=1)
            in_maps.append({
                "xT": xT[b],
                "wq": np.ascontiguousarray(
                    Wq[tp * DQ:(tp + 1) * DQ, :].T).astype(BF16NP),
                "wkv": np.ascontiguousarray(wkv).astype(BF16NP),
                "wo": np.ascontiguousarray(
                    Wo[:, tp * DQ:(tp + 1) * DQ].T).astype(BF16NP),
                "cst": cst,
                "qgb": qgb_row,
                "ident": ident,
                "ones": ones,
                "msk": msk,
            })
    return in_maps


def run(x, Wq, Wk, Wv, Wo, qg, trace=False, **trace_kwargs):
    nc = get_nc()
    in_maps = make_in_maps(x, Wq, Wk, Wv, Wo, qg)
    res = run_bass_kernel_spmd(nc, in_maps, core_ids=list(range(NCORES)),
                               trace=trace, **trace_kwargs)
    out = np.empty((NB, S, D), np.float32)
    for b in range(NB):
        acc = res.results[b * NTP]["outT"].astype(np.float32)
        for tp in range(1, NTP):
            acc = acc + res.results[b * NTP + tp]["outT"]
        out[b] = acc.T
    return out, res


def kernel(x, Wq, Wk, Wv, Wo, qg):
    out, _ = run(x, Wq, Wk, Wv, Wo, qg)
    return out

